# revision 25
# baseline (speedup 1.0000x reference)
"""GPTNeoX attention (B=1, S=2048, E=1024, 16 heads, hs=64) on 8 TRN2 cores.

Sharding: tensor-parallel across heads, 2 heads per core; host sums the 8
partial output projections (the all-reduce) and adds b_dense.

Perf notes vs the fp32 baseline (272us):
 - All matmuls run in bf16 (inputs pre-cast on host).  fp32r matmuls drew
   enough power to throttle the PE to 50% util for ~60% of the run; bf16
   streams at 1 col/cycle untrottled and halves LDWEIGHTS traffic.
 - rotary is folded into W_q on the host (W_q.T @ rot), removing the
   on-device fold matmuls + transposes.
 - b_v is folded into V *before* the PV matmul (per-partition add in the
   vT layout): P@(v+bv) = y_un + Z*bv, so the post-softmax normalize
   (y_un + Z*bv)/Z = y + bv needs no separate bias pass.
 - softmax denominator Z comes from a ones-column appended to V (row 64 of
   the PV accumulator); 1/Z via reciprocal_approx_fast (DVE) and the
   partition broadcast via gpsimd partition_broadcast — this replaces a
   1-partition reciprocal (6.5us) + 256KB broadcast DMA (11us) per head/qb.
 - phase-1 projections run ec-outer so matmuls start as soon as the first
   512KB xT chunk lands instead of after the full 4MB load.
"""

import numpy as np
import ml_dtypes

import concourse.bass as bass
import concourse.mybir as mybir
import concourse.tile as tile
from concourse import bacc
from concourse.bass_utils import run_bass_kernel_spmd

FP = mybir.dt.float32
BF = mybir.dt.bfloat16
AF = mybir.ActivationFunctionType

N_CORES = 8
E = 1024          # embed dim
S = 2048          # sequence
P = 128           # partitions
EO = E // P       # 8 e-chunks
HS = 64           # head size
NH_LOC = 2        # heads per core
SQB = 1024        # sq block (exp tile width, PSUM tile width)
NQB = S // SQB    # 2
SKC = S // P      # 16 sk chunks
NSC = S // P      # 16 s chunks for output


def build_nc():
    nc = bacc.Bacc("TRN2", target_bir_lowering=False, debug=False)

    xT_d = nc.dram_tensor("xT", (E, S), BF, kind="ExternalInput")
    wqT_d = nc.dram_tensor("wqT", (E, P), BF, kind="ExternalInput")
    wkT_d = nc.dram_tensor("wkT", (E, P), BF, kind="ExternalInput")
    wvT_d = nc.dram_tensor("wvT", (E, P), BF, kind="ExternalInput")
    wdT_d = nc.dram_tensor("wdT", (P, E), BF, kind="ExternalInput")
    bqe_d = nc.dram_tensor("bqe", (P,), FP, kind="ExternalInput")
    bk_d = nc.dram_tensor("bk", (P,), FP, kind="ExternalInput")
    bv_d = nc.dram_tensor("bv", (P,), FP, kind="ExternalInput")
    out_d = nc.dram_tensor("out", (S, E), BF, kind="ExternalOutput")

    xT_r = xT_d[:].rearrange("(eo p) s -> p eo s", p=P)
    wqT_r = wqT_d[:].rearrange("(eo p) g -> p eo g", p=P)
    wkT_r = wkT_d[:].rearrange("(eo p) g -> p eo g", p=P)
    wvT_r = wvT_d[:].rearrange("(eo p) g -> p eo g", p=P)

    with tile.TileContext(nc) as tc:
        with (
            nc.allow_low_precision(reason="bf16 matmul path; tol is 2e-2"),
            tc.tile_pool(name="const", bufs=1) as const,
            tc.tile_pool(name="work", bufs=3) as work,
            tc.tile_pool(name="nrm", bufs=2) as nrm,
            tc.tile_pool(name="outp", bufs=3) as outp,
            tc.tile_pool(name="psA", bufs=2, space="PSUM") as psA,
            tc.tile_pool(name="psB", bufs=2, space="PSUM") as psB,
            tc.tile_pool(name="drs", bufs=2, space="DRAM") as drs,
        ):
            # ---------- constant loads ----------
            # weights/biases issue on the scalar HWDGE queue, xT on the sync
            # queue, so the first projection matmul isn't serialized behind
            # 7 small-transfer issue latencies.
            wkT_sb = const.tile([P, EO, P], BF)
            nc.scalar.dma_start(wkT_sb[:], wkT_r[:])
            wqT_sb = const.tile([P, EO, P], BF)
            nc.scalar.dma_start(wqT_sb[:], wqT_r[:])
            wvT_sb = const.tile([P, EO, P], BF)
            nc.scalar.dma_start(wvT_sb[:], wvT_r[:])
            wdT_sb = const.tile([P, E], BF)
            nc.scalar.dma_start(wdT_sb[:], wdT_d[:])
            bqe_sb = const.tile([P, 1], FP)
            nc.scalar.dma_start(bqe_sb[:], bqe_d[:][:, None])
            bk_sb = const.tile([P, 1], FP)
            nc.scalar.dma_start(bk_sb[:], bk_d[:][:, None])
            bv_sb = const.tile([P, 1], FP)
            nc.scalar.dma_start(bv_sb[:], bv_d[:][:, None])
            xT_sb = const.tile([P, EO, S], BF)
            for eo in range(EO):
                eng = nc.sync if eo % 2 == 0 else nc.scalar
                eng.dma_start(xT_sb[:, eo, :], xT_r[:, eo, :])

            vaug_sb = const.tile([P, NH_LOC, SKC, HS + 1], BF)
            nc.gpsimd.memset(vaug_sb[:, :, :, HS:HS + 1], 1.0)

            qT_sb = const.tile([P, S], BF)
            kT_sb = const.tile([P, S], BF)
            vT_sb = const.tile([P, S], BF)
            yTn_sb = const.tile([P, S], BF)

            # ---------- phase 1: k/q projections (ec-outer, S halved) ----
            # kT[g,s] = sum_e wkT[e,g] xT[e,s] + bk[g]  (and q with folded
            # rotary weights + bias).  ec-outer overlaps with the xT DMA.
            for half in range(2):
                base = half * (S // 2)
                tk = psB.tile([P, SQB], FP, tag="yt")
                tq = psB.tile([P, SQB], FP, tag="yt")
                for ec in range(EO):
                    for (t, w) in ((tk, wkT_sb), (tq, wqT_sb)):
                        for r in range(2):
                            nc.tensor.matmul(
                                t[:, r * 512:(r + 1) * 512],
                                lhsT=w[:, ec, :],
                                rhs=xT_sb[:, ec, base + r * 512:
                                          base + (r + 1) * 512],
                                start=(ec == 0),
                                stop=(ec == EO - 1),
                            )
                for r in range(2):
                    sl = slice(base + r * 512, base + (r + 1) * 512)
                    nc.vector.tensor_scalar_add(
                        kT_sb[:, sl], tk[:, r * 512:(r + 1) * 512], bk_sb[:])
                    nc.vector.tensor_scalar_add(
                        qT_sb[:, sl], tq[:, r * 512:(r + 1) * 512], bqe_sb[:])

            # ---------- phase 2: v projection (+b_v) and transpose -------
            # vT[g,s] = sum_e wvT[e,g] xT[e,s] + bv[g]; then PE-transpose
            # 64x128 head-blocks into vaug[sk, d] (ones col preset above).
            for half in range(2):
                base = half * (S // 2)
                tv = psA.tile([P, SQB], FP, tag="st")
                for ec in range(EO):
                    for r in range(2):
                        nc.tensor.matmul(
                            tv[:, r * 512:(r + 1) * 512],
                            lhsT=wvT_sb[:, ec, :],
                            rhs=xT_sb[:, ec, base + r * 512:
                                      base + (r + 1) * 512],
                            start=(ec == 0),
                            stop=(ec == EO - 1),
                        )
                for r in range(2):
                    sl = slice(base + r * 512, base + (r + 1) * 512)
                    nc.vector.tensor_scalar_add(
                        vT_sb[:, sl], tv[:, r * 512:(r + 1) * 512], bv_sb[:])
            for h in range(NH_LOC):
                hsl = slice(h * HS, (h + 1) * HS)
                vstg = work.tile([P, SKC, HS], BF, tag="vstg")
                nc.sync.dma_start_transpose(vstg[:], vT_sb[hsl, :])
                nc.vector.tensor_copy(vaug_sb[:, h, :, :HS], vstg[:])

            # ---------- attention ----------
            # ST[sk,sq] = K Q^T / 8 -> P~ = exp; yt = [V+bv | 1]^T P~
            # y = yt[:64] * (1/Z) with Z = yt[64] (includes the Z*bv fold).
            # out[s,f] = sum_e yTn[e,s] wdT[e,f]: each qb's output projection
            # is interleaved into the NEXT qb's attention j-loop so the PE
            # never stalls on the (slow) softmax normalize chain.
            def emit_po(sc):
                po = psA.tile([P, SQB], FP, tag="st")
                for r in range(2):
                    rsl = slice(r * 512, (r + 1) * 512)
                    nc.tensor.matmul(
                        po[:, rsl],
                        lhsT=yTn_sb[:, sc * P:(sc + 1) * P],
                        rhs=wdT_sb[:, rsl],
                        start=True,
                        stop=True,
                    )
                ob = outp.tile([P, E], BF, tag="ob")
                if sc % 2 == 0:
                    nc.scalar.copy(ob[:], po[:])
                else:
                    nc.vector.tensor_copy(ob[:], po[:])
                nc.gpsimd.dma_start(out_d[sc * P:(sc + 1) * P, :], ob[:])

            # sq chunks: shrink toward the end so the final normalize +
            # outproj tail covers as little sequence as possible.
            chunks = [(0, 1024), (1024, 512), (1536, 512)]
            prev_po = []
            for (cq0, csz) in chunks:
                qsl = slice(cq0, cq0 + csz)
                for h in range(NH_LOC):
                    # previous chunk's outproj blocks, fed into this j-loop
                    # once its normalize has had time to finish
                    pending = prev_po if h == 0 else []
                    hsl = slice(h * HS, (h + 1) * HS)
                    yt = psB.tile([P, SQB], FP, tag="yt")
                    for j in range(SKC):
                        st = psA.tile([P, SQB], FP, tag="st")
                        for r in range(csz // 512):
                            rsl = slice(r * 512, (r + 1) * 512)
                            nc.tensor.matmul(
                                st[:, rsl],
                                lhsT=kT_sb[hsl, j * P:(j + 1) * P],
                                rhs=qT_sb[hsl, cq0 + r * 512:
                                          cq0 + (r + 1) * 512],
                                start=True,
                                stop=True,
                            )
                        pt = work.tile([P, SQB], BF, tag="pt")
                        nc.scalar.activation(
                            pt[:, :csz], st[:, :csz], AF.Exp, scale=0.125)
                        for r in range(csz // 512):
                            rsl = slice(r * 512, (r + 1) * 512)
                            nc.tensor.matmul(
                                yt[:HS + 1, rsl],
                                lhsT=vaug_sb[:, h, j, :],
                                rhs=pt[:, rsl],
                                start=(j == 0),
                                stop=(j == SKC - 1),
                            )
                        if j >= SKC - len(pending):
                            emit_po(pending[j - (SKC - len(pending))])
                    # normalize: y = yt[:64] / Z  (Z in row 64)
                    zri = nrm.tile([1, SQB], FP, tag="zri")
                    nc.vector.reciprocal(zri[:, :csz], yt[HS:HS + 1, :csz])
                    zrd = drs.tile([1, SQB], FP, tag="zrd")
                    nc.sync.dma_start(zrd[:, :csz], zri[:, :csz])
                    zbs = nrm.tile([HS, SQB], FP, tag="zbs")
                    nc.sync.dma_start(
                        zbs[:, :csz], zrd[0:1, :csz].to_broadcast((HS, csz)))
                    nc.vector.tensor_mul(
                        yTn_sb[hsl, qsl], yt[:HS, :csz], zbs[:, :csz])
                prev_po = list(range(cq0 // P, (cq0 + csz) // P))

            # last chunk's output projection is the tail
            for sc in prev_po:
                emit_po(sc)

    nc.compile()
    return nc


_NC_CACHE = None


def _get_nc():
    global _NC_CACHE
    if _NC_CACHE is None:
        _NC_CACHE = build_nc()
    return _NC_CACHE


def make_in_maps(x, W_qkv, b_qkv, rotary, W_dense, b_dense):
    x = np.asarray(x, dtype=np.float32)
    W_qkv = np.asarray(W_qkv, dtype=np.float32)
    b_qkv = np.asarray(b_qkv, dtype=np.float32)
    rotary = np.asarray(rotary, dtype=np.float32)
    W_dense = np.asarray(W_dense, dtype=np.float32)

    bf16 = ml_dtypes.bfloat16
    xT = np.ascontiguousarray(x.reshape(S, E).T.astype(bf16))
    wq = W_qkv[0:E, :]            # [E(out f), E(in e)]
    bq = b_qkv[0:E]
    in_maps = []
    for c in range(N_CORES):
        lo, hi = P * c, P * (c + 1)
        rot_c = rotary[:, lo:hi]                    # [E(f), 128(g)]
        wqT_eff = wq.T @ rot_c                      # [E(e), 128(g)]
        bqe = bq @ rot_c                            # [128(g)]
        in_maps.append({
            "xT": xT,
            "wqT": np.ascontiguousarray(wqT_eff.astype(bf16)),
            "wkT": np.ascontiguousarray(W_qkv[E + lo:E + hi, :].T.astype(bf16)),
            "wvT": np.ascontiguousarray(
                W_qkv[2 * E + lo:2 * E + hi, :].T.astype(bf16)),
            "wdT": np.ascontiguousarray(W_dense[:, lo:hi].T.astype(bf16)),
            "bqe": np.ascontiguousarray(bqe),
            "bk": np.ascontiguousarray(b_qkv[E + lo:E + hi]),
            "bv": np.ascontiguousarray(b_qkv[2 * E + lo:2 * E + hi]),
        })
    return in_maps


def run(inputs, trace=False, **trace_kwargs):
    """Run on 8 cores; returns (full_output, BassKernelResults)."""
    nc = _get_nc()
    in_maps = make_in_maps(**inputs)
    br = run_bass_kernel_spmd(
        nc, in_maps, core_ids=list(range(N_CORES)), trace=trace, **trace_kwargs
    )
    b_dense = np.asarray(inputs["b_dense"], dtype=np.float32)
    acc = np.zeros((S, E), dtype=np.float32)
    for r in br.results:
        acc += np.asarray(r["out"], dtype=np.float32)
    acc += b_dense[None, :]
    return acc[None, :, :], br


def kernel(**inputs) -> np.ndarray:
    out, _ = run(inputs, trace=False)
    return out


# revision 30
# speedup vs baseline: 1.0148x; 1.0148x over previous
"""GPTNeoX attention (B=1, S=2048, E=1024, 16 heads, hs=64) on 8 TRN2 cores.

Sharding: tensor-parallel across heads, 2 heads per core; host sums the 8
partial output projections (the all-reduce) and adds b_dense.

Perf notes vs the fp32 baseline (272us):
 - All matmuls run in bf16 (inputs pre-cast on host).  fp32r matmuls drew
   enough power to throttle the PE to 50% util for ~60% of the run; bf16
   streams at 1 col/cycle untrottled and halves LDWEIGHTS traffic.
 - rotary is folded into W_q on the host (W_q.T @ rot), removing the
   on-device fold matmuls + transposes.
 - b_v is folded into V *before* the PV matmul (per-partition add in the
   vT layout): P@(v+bv) = y_un + Z*bv, so the post-softmax normalize
   (y_un + Z*bv)/Z = y + bv needs no separate bias pass.
 - softmax denominator Z comes from a ones-column appended to V (row 64 of
   the PV accumulator); 1/Z via reciprocal_approx_fast (DVE) and the
   partition broadcast via gpsimd partition_broadcast — this replaces a
   1-partition reciprocal (6.5us) + 256KB broadcast DMA (11us) per head/qb.
 - phase-1 projections run ec-outer so matmuls start as soon as the first
   512KB xT chunk lands instead of after the full 4MB load.
"""

import numpy as np
import ml_dtypes

import concourse.bass as bass
import concourse.mybir as mybir
import concourse.tile as tile
from concourse import bacc
from concourse.bass_utils import run_bass_kernel_spmd

FP = mybir.dt.float32
BF = mybir.dt.bfloat16
AF = mybir.ActivationFunctionType

N_CORES = 8
E = 1024          # embed dim
S = 2048          # sequence
P = 128           # partitions
EO = E // P       # 8 e-chunks
HS = 64           # head size
NH_LOC = 2        # heads per core
SQB = 1024        # sq block (exp tile width, PSUM tile width)
NQB = S // SQB    # 2
SKC = S // P      # 16 sk chunks
NSC = S // P      # 16 s chunks for output


def build_nc():
    nc = bacc.Bacc("TRN2", target_bir_lowering=False, debug=False)

    xT_d = nc.dram_tensor("xT", (E, S), BF, kind="ExternalInput")
    wqT_d = nc.dram_tensor("wqT", (E, P), BF, kind="ExternalInput")
    wkT_d = nc.dram_tensor("wkT", (E, P), BF, kind="ExternalInput")
    wvT_d = nc.dram_tensor("wvT", (E, P), BF, kind="ExternalInput")
    wdT_d = nc.dram_tensor("wdT", (P, E), BF, kind="ExternalInput")
    bqe_d = nc.dram_tensor("bqe", (P,), FP, kind="ExternalInput")
    bk_d = nc.dram_tensor("bk", (P,), FP, kind="ExternalInput")
    bv_d = nc.dram_tensor("bv", (P,), FP, kind="ExternalInput")
    out_d = nc.dram_tensor("out", (S, E), BF, kind="ExternalOutput")

    xT_r = xT_d[:].rearrange("(eo p) s -> p eo s", p=P)
    wqT_r = wqT_d[:].rearrange("(eo p) g -> p eo g", p=P)
    wkT_r = wkT_d[:].rearrange("(eo p) g -> p eo g", p=P)
    wvT_r = wvT_d[:].rearrange("(eo p) g -> p eo g", p=P)

    with tile.TileContext(nc) as tc:
        with (
            nc.allow_low_precision(reason="bf16 matmul path; tol is 2e-2"),
            tc.tile_pool(name="const", bufs=1) as const,
            tc.tile_pool(name="work", bufs=3) as work,
            tc.tile_pool(name="nrm", bufs=2) as nrm,
            tc.tile_pool(name="outp", bufs=3) as outp,
            tc.tile_pool(name="psA", bufs=2, space="PSUM") as psA,
            tc.tile_pool(name="psB", bufs=2, space="PSUM") as psB,
            tc.tile_pool(name="drs", bufs=2, space="DRAM") as drs,
        ):
            # ---------- constant loads ----------
            # weights/biases issue on the scalar HWDGE queue, xT on the sync
            # queue, so the first projection matmul isn't serialized behind
            # 7 small-transfer issue latencies.
            wkT_sb = const.tile([P, EO, P], BF)
            nc.scalar.dma_start(wkT_sb[:], wkT_r[:])
            wqT_sb = const.tile([P, EO, P], BF)
            nc.scalar.dma_start(wqT_sb[:], wqT_r[:])
            wvT_sb = const.tile([P, EO, P], BF)
            nc.scalar.dma_start(wvT_sb[:], wvT_r[:])
            wdT_sb = const.tile([P, E], BF)
            nc.scalar.dma_start(wdT_sb[:], wdT_d[:])
            bqe_sb = const.tile([P, 1], FP)
            nc.scalar.dma_start(bqe_sb[:], bqe_d[:][:, None])
            bk_sb = const.tile([P, 1], FP)
            nc.scalar.dma_start(bk_sb[:], bk_d[:][:, None])
            bv_sb = const.tile([P, 1], FP)
            nc.scalar.dma_start(bv_sb[:], bv_d[:][:, None])
            xT_sb = const.tile([P, EO, S], BF)
            # eo0 split so the first projection matmul (which only needs
            # its first 512 columns) starts as early as possible
            nc.sync.dma_start(xT_sb[:, 0, :1024], xT_r[:, 0, :1024])
            nc.sync.dma_start(xT_sb[:, 0, 1024:], xT_r[:, 0, 1024:])
            for eo in range(1, EO):
                eng = nc.sync if eo % 2 == 0 else nc.scalar
                eng.dma_start(xT_sb[:, eo, :], xT_r[:, eo, :])

            vaug_sb = const.tile([P, NH_LOC, SKC, HS + 1], BF)
            nc.gpsimd.memset(vaug_sb[:, :, :, HS:HS + 1], 1.0)

            qT_sb = const.tile([P, S], BF)
            kT_sb = const.tile([P, S], BF)
            vT_sb = const.tile([P, S], BF)
            yTn_sb = const.tile([P, S], BF)

            # ---------- phase 1: k/q projections (ec-outer, S halved) ----
            # kT[g,s] = sum_e wkT[e,g] xT[e,s] + bk[g]  (and q with folded
            # rotary weights + bias).  ec-outer overlaps with the xT DMA.
            for half in range(2):
                base = half * (S // 2)
                tk = psB.tile([P, SQB], FP, tag="yt")
                tq = psB.tile([P, SQB], FP, tag="yt")
                for ec in range(EO):
                    for (t, w) in ((tk, wkT_sb), (tq, wqT_sb)):
                        for r in range(2):
                            nc.tensor.matmul(
                                t[:, r * 512:(r + 1) * 512],
                                lhsT=w[:, ec, :],
                                rhs=xT_sb[:, ec, base + r * 512:
                                          base + (r + 1) * 512],
                                start=(ec == 0),
                                stop=(ec == EO - 1),
                            )
                for r in range(2):
                    sl = slice(base + r * 512, base + (r + 1) * 512)
                    nc.vector.tensor_scalar_add(
                        kT_sb[:, sl], tk[:, r * 512:(r + 1) * 512], bk_sb[:])
                    nc.vector.tensor_scalar_add(
                        qT_sb[:, sl], tq[:, r * 512:(r + 1) * 512], bqe_sb[:])

            # ---------- phase 2: v projection (+b_v) and transpose -------
            # vT[g,s] = sum_e wvT[e,g] xT[e,s] + bv[g]; then PE-transpose
            # 64x128 head-blocks into vaug[sk, d] (ones col preset above).
            for half in range(2):
                base = half * (S // 2)
                tv = psA.tile([P, SQB], FP, tag="st")
                for ec in range(EO):
                    for r in range(2):
                        nc.tensor.matmul(
                            tv[:, r * 512:(r + 1) * 512],
                            lhsT=wvT_sb[:, ec, :],
                            rhs=xT_sb[:, ec, base + r * 512:
                                      base + (r + 1) * 512],
                            start=(ec == 0),
                            stop=(ec == EO - 1),
                        )
                for r in range(2):
                    sl = slice(base + r * 512, base + (r + 1) * 512)
                    nc.vector.tensor_scalar_add(
                        vT_sb[:, sl], tv[:, r * 512:(r + 1) * 512], bv_sb[:])
            for h in range(NH_LOC):
                hsl = slice(h * HS, (h + 1) * HS)
                vstg = work.tile([P, SKC, HS], BF, tag="vstg")
                nc.sync.dma_start_transpose(vstg[:], vT_sb[hsl, :])
                nc.vector.tensor_copy(vaug_sb[:, h, :, :HS], vstg[:])

            # ---------- attention ----------
            # ST[sk,sq] = K Q^T / 8 -> P~ = exp; yt = [V+bv | 1]^T P~
            # y = yt[:64] * (1/Z) with Z = yt[64] (includes the Z*bv fold).
            # out[s,f] = sum_e yTn[e,s] wdT[e,f]: each qb's output projection
            # is interleaved into the NEXT qb's attention j-loop so the PE
            # never stalls on the (slow) softmax normalize chain.
            def emit_po(sc):
                po = psA.tile([P, SQB], FP, tag="st")
                for r in range(2):
                    rsl = slice(r * 512, (r + 1) * 512)
                    nc.tensor.matmul(
                        po[:, rsl],
                        lhsT=yTn_sb[:, sc * P:(sc + 1) * P],
                        rhs=wdT_sb[:, rsl],
                        start=True,
                        stop=True,
                    )
                ob = outp.tile([P, E], BF, tag="ob")
                if sc % 2 == 0:
                    nc.scalar.copy(ob[:], po[:])
                else:
                    nc.vector.tensor_copy(ob[:], po[:])
                nc.gpsimd.dma_start(out_d[sc * P:(sc + 1) * P, :], ob[:])

            # sq chunks: shrink toward the end so the final normalize +
            # outproj tail covers as little sequence as possible.
            chunks = [(0, 1024), (1024, 1024)]
            prev_po = []
            for (cq0, csz) in chunks:
                qsl = slice(cq0, cq0 + csz)
                for h in range(NH_LOC):
                    # previous chunk's outproj blocks, fed into this j-loop
                    # once its normalize has had time to finish
                    pending = prev_po if h == 0 else []
                    hsl = slice(h * HS, (h + 1) * HS)
                    yt = psB.tile([P, SQB], FP, tag="yt")
                    for j in range(SKC):
                        st = psA.tile([P, SQB], FP, tag="st")
                        for r in range(csz // 512):
                            rsl = slice(r * 512, (r + 1) * 512)
                            nc.tensor.matmul(
                                st[:, rsl],
                                lhsT=kT_sb[hsl, j * P:(j + 1) * P],
                                rhs=qT_sb[hsl, cq0 + r * 512:
                                          cq0 + (r + 1) * 512],
                                start=True,
                                stop=True,
                            )
                        pt = work.tile([P, SQB], BF, tag="pt")
                        nc.scalar.activation(
                            pt[:, :csz], st[:, :csz], AF.Exp, scale=0.125)
                        for r in range(csz // 512):
                            rsl = slice(r * 512, (r + 1) * 512)
                            nc.tensor.matmul(
                                yt[:HS + 1, rsl],
                                lhsT=vaug_sb[:, h, j, :],
                                rhs=pt[:, rsl],
                                start=(j == 0),
                                stop=(j == SKC - 1),
                            )
                        if j >= SKC - len(pending):
                            emit_po(pending[j - (SKC - len(pending))])
                    # normalize: y = yt[:64] / Z  (Z in row 64); 1/Z is DMA
                    # round-trip broadcast across partitions.  The very last
                    # (chunk, h) is the kernel tail: split it into pipelined
                    # halves and chase each half with its outproj blocks so
                    # the PE restarts as early as possible.
                    last = (cq0, h) == (chunks[-1][0], NH_LOC - 1)
                    nsp = 2 if last else 1
                    for half in range(nsp):
                        w = csz // nsp
                        lsl = slice(half * w, (half + 1) * w)
                        gsl = slice(cq0 + half * w, cq0 + (half + 1) * w)
                        zri = nrm.tile([1, SQB], FP, tag="zri")
                        nc.vector.reciprocal(
                            zri[:, lsl], yt[HS:HS + 1, lsl])
                        zrd = drs.tile([1, SQB], FP, tag="zrd")
                        nc.sync.dma_start(zrd[:, lsl], zri[:, lsl])
                        zbs = nrm.tile([HS, SQB], FP, tag="zbs")
                        nc.sync.dma_start(
                            zbs[:, lsl],
                            zrd[0:1, lsl].to_broadcast((HS, w)))
                        nc.vector.tensor_mul(
                            yTn_sb[hsl, gsl], yt[:HS, lsl], zbs[:, lsl])
                        if last:
                            for sc in range((cq0 + half * w) // P,
                                            (cq0 + (half + 1) * w) // P):
                                emit_po(sc)
                prev_po = list(range(cq0 // P, (cq0 + csz) // P))

    nc.compile()
    return nc


_NC_CACHE = None


def _get_nc():
    global _NC_CACHE
    if _NC_CACHE is None:
        _NC_CACHE = build_nc()
    return _NC_CACHE


def make_in_maps(x, W_qkv, b_qkv, rotary, W_dense, b_dense):
    x = np.asarray(x, dtype=np.float32)
    W_qkv = np.asarray(W_qkv, dtype=np.float32)
    b_qkv = np.asarray(b_qkv, dtype=np.float32)
    rotary = np.asarray(rotary, dtype=np.float32)
    W_dense = np.asarray(W_dense, dtype=np.float32)

    bf16 = ml_dtypes.bfloat16
    xT = np.ascontiguousarray(x.reshape(S, E).T.astype(bf16))
    wq = W_qkv[0:E, :]            # [E(out f), E(in e)]
    bq = b_qkv[0:E]
    in_maps = []
    for c in range(N_CORES):
        lo, hi = P * c, P * (c + 1)
        rot_c = rotary[:, lo:hi]                    # [E(f), 128(g)]
        wqT_eff = wq.T @ rot_c                      # [E(e), 128(g)]
        bqe = bq @ rot_c                            # [128(g)]
        in_maps.append({
            "xT": xT,
            "wqT": np.ascontiguousarray(wqT_eff.astype(bf16)),
            "wkT": np.ascontiguousarray(W_qkv[E + lo:E + hi, :].T.astype(bf16)),
            "wvT": np.ascontiguousarray(
                W_qkv[2 * E + lo:2 * E + hi, :].T.astype(bf16)),
            "wdT": np.ascontiguousarray(W_dense[:, lo:hi].T.astype(bf16)),
            "bqe": np.ascontiguousarray(bqe),
            "bk": np.ascontiguousarray(b_qkv[E + lo:E + hi]),
            "bv": np.ascontiguousarray(b_qkv[2 * E + lo:2 * E + hi]),
        })
    return in_maps


def run(inputs, trace=False, **trace_kwargs):
    """Run on 8 cores; returns (full_output, BassKernelResults)."""
    nc = _get_nc()
    in_maps = make_in_maps(**inputs)
    br = run_bass_kernel_spmd(
        nc, in_maps, core_ids=list(range(N_CORES)), trace=trace, **trace_kwargs
    )
    b_dense = np.asarray(inputs["b_dense"], dtype=np.float32)
    acc = np.zeros((S, E), dtype=np.float32)
    for r in br.results:
        acc += np.asarray(r["out"], dtype=np.float32)
    acc += b_dense[None, :]
    return acc[None, :, :], br


def kernel(**inputs) -> np.ndarray:
    out, _ = run(inputs, trace=False)
    return out


# revision 32
# speedup vs baseline: 1.0815x; 1.0657x over previous
"""GPTNeoX attention (B=1, S=2048, E=1024, 16 heads, hs=64) on 8 TRN2 cores.

Sharding: tensor-parallel across heads, 2 heads per core; host sums the 8
partial output projections (the all-reduce) and adds b_dense.

Perf notes vs the fp32 baseline (272us):
 - All matmuls run in bf16 (inputs pre-cast on host).  fp32r matmuls drew
   enough power to throttle the PE to 50% util for ~60% of the run; bf16
   streams at 1 col/cycle untrottled and halves LDWEIGHTS traffic.
 - rotary is folded into W_q on the host (W_q.T @ rot), removing the
   on-device fold matmuls + transposes.
 - b_v is folded into V *before* the PV matmul (per-partition add in the
   vT layout): P@(v+bv) = y_un + Z*bv, so the post-softmax normalize
   (y_un + Z*bv)/Z = y + bv needs no separate bias pass.
 - softmax denominator Z comes from a ones-column appended to V (row 64 of
   the PV accumulator); 1/Z via reciprocal_approx_fast (DVE) and the
   partition broadcast via gpsimd partition_broadcast — this replaces a
   1-partition reciprocal (6.5us) + 256KB broadcast DMA (11us) per head/qb.
 - phase-1 projections run ec-outer so matmuls start as soon as the first
   512KB xT chunk lands instead of after the full 4MB load.
"""

import numpy as np
import ml_dtypes

import concourse.bass as bass
import concourse.mybir as mybir
import concourse.tile as tile
from concourse import bacc
from concourse.bass_utils import run_bass_kernel_spmd

FP = mybir.dt.float32
BF = mybir.dt.bfloat16
AF = mybir.ActivationFunctionType

N_CORES = 8
E = 1024          # embed dim
S = 2048          # sequence
P = 128           # partitions
EO = E // P       # 8 e-chunks
HS = 64           # head size
NH_LOC = 2        # heads per core
SQB = 1024        # sq block (exp tile width, PSUM tile width)
NQB = S // SQB    # 2
SKC = S // P      # 16 sk chunks
NSC = S // P      # 16 s chunks for output


def build_nc():
    nc = bacc.Bacc("TRN2", target_bir_lowering=False, debug=False)

    xT_d = nc.dram_tensor("xT", (E, S), BF, kind="ExternalInput")
    wqT_d = nc.dram_tensor("wqT", (E, P), BF, kind="ExternalInput")
    wkT_d = nc.dram_tensor("wkT", (E, P), BF, kind="ExternalInput")
    wvT_d = nc.dram_tensor("wvT", (E, P), BF, kind="ExternalInput")
    wdT_d = nc.dram_tensor("wdT", (P, E), BF, kind="ExternalInput")
    bqe_d = nc.dram_tensor("bqe", (P,), FP, kind="ExternalInput")
    bk_d = nc.dram_tensor("bk", (P,), FP, kind="ExternalInput")
    bv_d = nc.dram_tensor("bv", (P,), FP, kind="ExternalInput")
    out_d = nc.dram_tensor("out", (S, E), BF, kind="ExternalOutput")

    xT_r = xT_d[:].rearrange("(eo p) s -> p eo s", p=P)
    wqT_r = wqT_d[:].rearrange("(eo p) g -> p eo g", p=P)
    wkT_r = wkT_d[:].rearrange("(eo p) g -> p eo g", p=P)
    wvT_r = wvT_d[:].rearrange("(eo p) g -> p eo g", p=P)

    with tile.TileContext(nc) as tc:
        with (
            nc.allow_low_precision(reason="bf16 matmul path; tol is 2e-2"),
            tc.tile_pool(name="const", bufs=1) as const,
            tc.tile_pool(name="work", bufs=3) as work,
            tc.tile_pool(name="nrm", bufs=2) as nrm,
            tc.tile_pool(name="outp", bufs=3) as outp,
            tc.tile_pool(name="psA", bufs=2, space="PSUM") as psA,
            tc.tile_pool(name="psB", bufs=2, space="PSUM") as psB,
            tc.tile_pool(name="drs", bufs=2, space="DRAM") as drs,
        ):
            # ---------- constant loads ----------
            # weights/biases issue on the scalar HWDGE queue, xT on the sync
            # queue, so the first projection matmul isn't serialized behind
            # 7 small-transfer issue latencies.
            wkT_sb = const.tile([P, EO, P], BF)
            nc.scalar.dma_start(wkT_sb[:], wkT_r[:])
            wqT_sb = const.tile([P, EO, P], BF)
            nc.scalar.dma_start(wqT_sb[:], wqT_r[:])
            wvT_sb = const.tile([P, EO, P], BF)
            nc.scalar.dma_start(wvT_sb[:], wvT_r[:])
            wdT_sb = const.tile([P, E], BF)
            nc.scalar.dma_start(wdT_sb[:], wdT_d[:])
            bqe_sb = const.tile([P, 1], FP)
            nc.scalar.dma_start(bqe_sb[:], bqe_d[:][:, None])
            bk_sb = const.tile([P, 1], FP)
            nc.scalar.dma_start(bk_sb[:], bk_d[:][:, None])
            bv_sb = const.tile([P, 1], FP)
            nc.scalar.dma_start(bv_sb[:], bv_d[:][:, None])
            xT_sb = const.tile([P, EO, S], BF)
            # eo0 split so the first projection matmul (which only needs
            # its first 512 columns) starts as early as possible
            nc.sync.dma_start(xT_sb[:, 0, :1024], xT_r[:, 0, :1024])
            nc.sync.dma_start(xT_sb[:, 0, 1024:], xT_r[:, 0, 1024:])
            for eo in range(1, EO):
                eng = nc.sync if eo % 2 == 0 else nc.scalar
                eng.dma_start(xT_sb[:, eo, :], xT_r[:, eo, :])

            vaug_sb = const.tile([P, NH_LOC, SKC, HS + 1], BF)
            nc.gpsimd.memset(vaug_sb[:, :, :, HS:HS + 1], 1.0)

            qT_sb = const.tile([P, S], BF)
            kT_sb = const.tile([P, S], BF)
            vT_sb = const.tile([P, S], BF)
            yTn_sb = const.tile([P, S], BF)

            # ---------- phase 1: k/q projections (ec-outer, S halved) ----
            # kT[g,s] = sum_e wkT[e,g] xT[e,s] + bk[g]  (and q with folded
            # rotary weights + bias).  ec-outer overlaps with the xT DMA.
            for half in range(2):
                base = half * (S // 2)
                tk = psB.tile([P, SQB], FP, tag="yt")
                tq = psB.tile([P, SQB], FP, tag="yt")
                for ec in range(EO):
                    for (t, w) in ((tk, wkT_sb), (tq, wqT_sb)):
                        for r in range(2):
                            nc.tensor.matmul(
                                t[:, r * 512:(r + 1) * 512],
                                lhsT=w[:, ec, :],
                                rhs=xT_sb[:, ec, base + r * 512:
                                          base + (r + 1) * 512],
                                start=(ec == 0),
                                stop=(ec == EO - 1),
                            )
                for r in range(2):
                    sl = slice(base + r * 512, base + (r + 1) * 512)
                    nc.vector.tensor_scalar_add(
                        kT_sb[:, sl], tk[:, r * 512:(r + 1) * 512], bk_sb[:])
                    nc.vector.tensor_scalar_add(
                        qT_sb[:, sl], tq[:, r * 512:(r + 1) * 512], bqe_sb[:])

            # ---------- phase 2: v projection (+b_v) and transpose -------
            # vT[g,s] = sum_e wvT[e,g] xT[e,s] + bv[g]; then PE-transpose
            # 64x128 head-blocks into vaug[sk, d] (ones col preset above).
            for half in range(2):
                base = half * (S // 2)
                tv = psA.tile([P, SQB], FP, tag="st")
                for ec in range(EO):
                    for r in range(2):
                        nc.tensor.matmul(
                            tv[:, r * 512:(r + 1) * 512],
                            lhsT=wvT_sb[:, ec, :],
                            rhs=xT_sb[:, ec, base + r * 512:
                                      base + (r + 1) * 512],
                            start=(ec == 0),
                            stop=(ec == EO - 1),
                        )
                for r in range(2):
                    sl = slice(base + r * 512, base + (r + 1) * 512)
                    nc.vector.tensor_scalar_add(
                        vT_sb[:, sl], tv[:, r * 512:(r + 1) * 512], bv_sb[:])
            for h in range(NH_LOC):
                hsl = slice(h * HS, (h + 1) * HS)
                vstg = work.tile([P, SKC, HS], BF, tag="vstg")
                nc.sync.dma_start_transpose(vstg[:], vT_sb[hsl, :])
                nc.vector.tensor_copy(vaug_sb[:, h, :, :HS], vstg[:])

            # ---------- attention ----------
            # ST[sk,sq] = K Q^T / 8 -> P~ = exp; yt = [V+bv | 1]^T P~
            # y = yt[:64] * (1/Z) with Z = yt[64] (includes the Z*bv fold).
            # out[s,f] = sum_e yTn[e,s] wdT[e,f]: each qb's output projection
            # is interleaved into the NEXT qb's attention j-loop so the PE
            # never stalls on the (slow) softmax normalize chain.
            def emit_po(sc, tail=False):
                po = psA.tile([P, SQB], FP, tag="st")
                for r in range(2):
                    rsl = slice(r * 512, (r + 1) * 512)
                    nc.tensor.matmul(
                        po[:, rsl],
                        lhsT=yTn_sb[:, sc * P:(sc + 1) * P],
                        rhs=wdT_sb[:, rsl],
                        start=True,
                        stop=True,
                    )
                ob = outp.tile([P, E], BF, tag="ob")
                if sc % 2 == 0:
                    nc.scalar.copy(ob[:], po[:])
                else:
                    nc.vector.tensor_copy(ob[:], po[:])
                eng = nc.sync if tail else nc.gpsimd
                eng.dma_start(out_d[sc * P:(sc + 1) * P, :], ob[:])

            # sq chunks: shrink toward the end so the final normalize +
            # outproj tail covers as little sequence as possible.
            chunks = [(0, 1024), (1024, 1024)]
            prev_po = []
            for (cq0, csz) in chunks:
                qsl = slice(cq0, cq0 + csz)
                for h in range(NH_LOC):
                    # previous chunk's outproj blocks, fed into this j-loop
                    # once its normalize has had time to finish
                    pending = prev_po if h == 0 else []
                    hsl = slice(h * HS, (h + 1) * HS)
                    yt = psB.tile([P, SQB], FP, tag="yt")
                    for j in range(SKC):
                        st = psA.tile([P, SQB], FP, tag="st")
                        for r in range(csz // 512):
                            rsl = slice(r * 512, (r + 1) * 512)
                            nc.tensor.matmul(
                                st[:, rsl],
                                lhsT=kT_sb[hsl, j * P:(j + 1) * P],
                                rhs=qT_sb[hsl, cq0 + r * 512:
                                          cq0 + (r + 1) * 512],
                                start=True,
                                stop=True,
                            )
                        pt = work.tile([P, SQB], BF, tag="pt")
                        nc.scalar.activation(
                            pt[:, :csz], st[:, :csz], AF.Exp, scale=0.125)
                        for r in range(csz // 512):
                            rsl = slice(r * 512, (r + 1) * 512)
                            nc.tensor.matmul(
                                yt[:HS + 1, rsl],
                                lhsT=vaug_sb[:, h, j, :],
                                rhs=pt[:, rsl],
                                start=(j == 0),
                                stop=(j == SKC - 1),
                            )
                        if j >= SKC - len(pending):
                            emit_po(pending[j - (SKC - len(pending))])
                    # normalize: y = yt[:64] / Z  (Z in row 64); 1/Z is DMA
                    # round-trip broadcast across partitions.  The very last
                    # (chunk, h) is the kernel tail: split it into pipelined
                    # halves and chase each half with its outproj blocks so
                    # the PE restarts as early as possible.
                    last = (cq0, h) == (chunks[-1][0], NH_LOC - 1)
                    nsp = 2 if last else 1
                    for half in range(nsp):
                        w = csz // nsp
                        lsl = slice(half * w, (half + 1) * w)
                        gsl = slice(cq0 + half * w, cq0 + (half + 1) * w)
                        zri = nrm.tile([1, SQB], FP, tag="zri")
                        nc.vector.reciprocal(
                            zri[:, lsl], yt[HS:HS + 1, lsl])
                        zrd = drs.tile([1, SQB], FP, tag="zrd")
                        nc.sync.dma_start(zrd[:, lsl], zri[:, lsl])
                        zbs = nrm.tile([HS, SQB], FP, tag="zbs")
                        nc.sync.dma_start(
                            zbs[:, lsl],
                            zrd[0:1, lsl].to_broadcast((HS, w)))
                        nc.vector.tensor_mul(
                            yTn_sb[hsl, gsl], yt[:HS, lsl], zbs[:, lsl])
                        if last:
                            for sc in range((cq0 + half * w) // P,
                                            (cq0 + (half + 1) * w) // P):
                                emit_po(sc, tail=True)
                prev_po = list(range(cq0 // P, (cq0 + csz) // P))

    nc.compile()
    return nc


_NC_CACHE = None


def _get_nc():
    global _NC_CACHE
    if _NC_CACHE is None:
        _NC_CACHE = build_nc()
    return _NC_CACHE


def make_in_maps(x, W_qkv, b_qkv, rotary, W_dense, b_dense):
    x = np.asarray(x, dtype=np.float32)
    W_qkv = np.asarray(W_qkv, dtype=np.float32)
    b_qkv = np.asarray(b_qkv, dtype=np.float32)
    rotary = np.asarray(rotary, dtype=np.float32)
    W_dense = np.asarray(W_dense, dtype=np.float32)

    bf16 = ml_dtypes.bfloat16
    xT = np.ascontiguousarray(x.reshape(S, E).T.astype(bf16))
    wq = W_qkv[0:E, :]            # [E(out f), E(in e)]
    bq = b_qkv[0:E]
    in_maps = []
    for c in range(N_CORES):
        lo, hi = P * c, P * (c + 1)
        rot_c = rotary[:, lo:hi]                    # [E(f), 128(g)]
        wqT_eff = wq.T @ rot_c                      # [E(e), 128(g)]
        bqe = bq @ rot_c                            # [128(g)]
        in_maps.append({
            "xT": xT,
            "wqT": np.ascontiguousarray(wqT_eff.astype(bf16)),
            "wkT": np.ascontiguousarray(W_qkv[E + lo:E + hi, :].T.astype(bf16)),
            "wvT": np.ascontiguousarray(
                W_qkv[2 * E + lo:2 * E + hi, :].T.astype(bf16)),
            "wdT": np.ascontiguousarray(W_dense[:, lo:hi].T.astype(bf16)),
            "bqe": np.ascontiguousarray(bqe),
            "bk": np.ascontiguousarray(b_qkv[E + lo:E + hi]),
            "bv": np.ascontiguousarray(b_qkv[2 * E + lo:2 * E + hi]),
        })
    return in_maps


def run(inputs, trace=False, **trace_kwargs):
    """Run on 8 cores; returns (full_output, BassKernelResults)."""
    nc = _get_nc()
    in_maps = make_in_maps(**inputs)
    br = run_bass_kernel_spmd(
        nc, in_maps, core_ids=list(range(N_CORES)), trace=trace, **trace_kwargs
    )
    b_dense = np.asarray(inputs["b_dense"], dtype=np.float32)
    acc = np.zeros((S, E), dtype=np.float32)
    for r in br.results:
        acc += np.asarray(r["out"], dtype=np.float32)
    acc += b_dense[None, :]
    return acc[None, :, :], br


def kernel(**inputs) -> np.ndarray:
    out, _ = run(inputs, trace=False)
    return out


# revision 34
# speedup vs baseline: 1.1047x; 1.0214x over previous
"""GPTNeoX attention (B=1, S=2048, E=1024, 16 heads, hs=64) on 8 TRN2 cores.

Sharding: tensor-parallel across heads, 2 heads per core; host sums the 8
partial output projections (the all-reduce) and adds b_dense.

Perf notes vs the fp32 baseline (272us):
 - All matmuls run in bf16 (inputs pre-cast on host).  fp32r matmuls drew
   enough power to throttle the PE to 50% util for ~60% of the run; bf16
   streams at 1 col/cycle untrottled and halves LDWEIGHTS traffic.
 - rotary is folded into W_q on the host (W_q.T @ rot), removing the
   on-device fold matmuls + transposes.
 - b_v is folded into V *before* the PV matmul (per-partition add in the
   vT layout): P@(v+bv) = y_un + Z*bv, so the post-softmax normalize
   (y_un + Z*bv)/Z = y + bv needs no separate bias pass.
 - softmax denominator Z comes from a ones-column appended to V (row 64 of
   the PV accumulator); 1/Z via reciprocal_approx_fast (DVE) and the
   partition broadcast via gpsimd partition_broadcast — this replaces a
   1-partition reciprocal (6.5us) + 256KB broadcast DMA (11us) per head/qb.
 - phase-1 projections run ec-outer so matmuls start as soon as the first
   512KB xT chunk lands instead of after the full 4MB load.
"""

import numpy as np
import ml_dtypes

import concourse.bass as bass
import concourse.mybir as mybir
import concourse.tile as tile
from concourse import bacc
from concourse.bass_utils import run_bass_kernel_spmd

FP = mybir.dt.float32
BF = mybir.dt.bfloat16
AF = mybir.ActivationFunctionType

N_CORES = 8
E = 1024          # embed dim
S = 2048          # sequence
P = 128           # partitions
EO = E // P       # 8 e-chunks
HS = 64           # head size
NH_LOC = 2        # heads per core
SQB = 1024        # sq block (exp tile width, PSUM tile width)
NQB = S // SQB    # 2
SKC = S // P      # 16 sk chunks
NSC = S // P      # 16 s chunks for output


def build_nc():
    nc = bacc.Bacc("TRN2", target_bir_lowering=False, debug=False)

    xT_d = nc.dram_tensor("xT", (E, S), BF, kind="ExternalInput")
    wqT_d = nc.dram_tensor("wqT", (E, P), BF, kind="ExternalInput")
    wkT_d = nc.dram_tensor("wkT", (E, P), BF, kind="ExternalInput")
    wvT_d = nc.dram_tensor("wvT", (E, P), BF, kind="ExternalInput")
    wdT_d = nc.dram_tensor("wdT", (P, E), BF, kind="ExternalInput")
    bqe_d = nc.dram_tensor("bqe", (P,), FP, kind="ExternalInput")
    bk_d = nc.dram_tensor("bk", (P,), FP, kind="ExternalInput")
    bv_d = nc.dram_tensor("bv", (P,), FP, kind="ExternalInput")
    out_d = nc.dram_tensor("out", (S, E), BF, kind="ExternalOutput")

    xT_r = xT_d[:].rearrange("(eo p) s -> p eo s", p=P)
    wqT_r = wqT_d[:].rearrange("(eo p) g -> p eo g", p=P)
    wkT_r = wkT_d[:].rearrange("(eo p) g -> p eo g", p=P)
    wvT_r = wvT_d[:].rearrange("(eo p) g -> p eo g", p=P)

    with tile.TileContext(nc) as tc:
        with (
            nc.allow_low_precision(reason="bf16 matmul path; tol is 2e-2"),
            tc.tile_pool(name="const", bufs=1) as const,
            tc.tile_pool(name="work", bufs=3) as work,
            tc.tile_pool(name="nrm", bufs=2) as nrm,
            tc.tile_pool(name="outp", bufs=3) as outp,
            tc.tile_pool(name="psA", bufs=2, space="PSUM") as psA,
            tc.tile_pool(name="psB", bufs=2, space="PSUM") as psB,
            tc.tile_pool(name="drs", bufs=2, space="DRAM") as drs,
        ):
            # ---------- constant loads ----------
            # weights/biases issue on the scalar HWDGE queue, xT on the sync
            # queue, so the first projection matmul isn't serialized behind
            # 7 small-transfer issue latencies.
            wkT_sb = const.tile([P, EO, P], BF)
            nc.scalar.dma_start(wkT_sb[:], wkT_r[:])
            wqT_sb = const.tile([P, EO, P], BF)
            nc.scalar.dma_start(wqT_sb[:], wqT_r[:])
            xT_sb = const.tile([P, EO, S], BF)
            # eo0 split so the first projection matmul (which only needs
            # its first 512 columns) starts as early as possible; odd
            # chunks go on the scalar queue right after the k/q weights,
            # in the order the ec-outer loop consumes them.
            nc.sync.dma_start(xT_sb[:, 0, :1024], xT_r[:, 0, :1024])
            nc.sync.dma_start(xT_sb[:, 0, 1024:], xT_r[:, 0, 1024:])
            for eo in (1, 3):
                nc.scalar.dma_start(xT_sb[:, eo, :], xT_r[:, eo, :])
            for eo in (2, 4, 6):
                nc.sync.dma_start(xT_sb[:, eo, :], xT_r[:, eo, :])
            wvT_sb = const.tile([P, EO, P], BF)
            nc.scalar.dma_start(wvT_sb[:], wvT_r[:])
            for eo in (5, 7):
                nc.scalar.dma_start(xT_sb[:, eo, :], xT_r[:, eo, :])
            wdT_sb = const.tile([P, E], BF)
            nc.scalar.dma_start(wdT_sb[:], wdT_d[:])
            bqe_sb = const.tile([P, 1], FP)
            nc.scalar.dma_start(bqe_sb[:], bqe_d[:][:, None])
            bk_sb = const.tile([P, 1], FP)
            nc.scalar.dma_start(bk_sb[:], bk_d[:][:, None])
            bv_sb = const.tile([P, 1], FP)
            nc.scalar.dma_start(bv_sb[:], bv_d[:][:, None])

            vaug_sb = const.tile([P, NH_LOC, SKC, HS + 1], BF)
            nc.gpsimd.memset(vaug_sb[:, :, :, HS:HS + 1], 1.0)

            qT_sb = const.tile([P, S], BF)
            kT_sb = const.tile([P, S], BF)
            vT_sb = const.tile([P, S], BF)
            yTn_sb = const.tile([P, S], BF)

            # ---------- phase 1: k/q projections (ec-outer, S halved) ----
            # kT[g,s] = sum_e wkT[e,g] xT[e,s] + bk[g]  (and q with folded
            # rotary weights + bias).  ec-outer overlaps with the xT DMA.
            for half in range(2):
                base = half * (S // 2)
                tk = psB.tile([P, SQB], FP, tag="yt")
                tq = psB.tile([P, SQB], FP, tag="yt")
                for ec in range(EO):
                    for (t, w) in ((tk, wkT_sb), (tq, wqT_sb)):
                        for r in range(2):
                            nc.tensor.matmul(
                                t[:, r * 512:(r + 1) * 512],
                                lhsT=w[:, ec, :],
                                rhs=xT_sb[:, ec, base + r * 512:
                                          base + (r + 1) * 512],
                                start=(ec == 0),
                                stop=(ec == EO - 1),
                            )
                for r in range(2):
                    sl = slice(base + r * 512, base + (r + 1) * 512)
                    nc.vector.tensor_scalar_add(
                        kT_sb[:, sl], tk[:, r * 512:(r + 1) * 512], bk_sb[:])
                    nc.vector.tensor_scalar_add(
                        qT_sb[:, sl], tq[:, r * 512:(r + 1) * 512], bqe_sb[:])

            # ---------- phase 2: v projection (+b_v) and transpose -------
            # vT[g,s] = sum_e wvT[e,g] xT[e,s] + bv[g]; then PE-transpose
            # 64x128 head-blocks into vaug[sk, d] (ones col preset above).
            for half in range(2):
                base = half * (S // 2)
                tv = psA.tile([P, SQB], FP, tag="st")
                for ec in range(EO):
                    for r in range(2):
                        nc.tensor.matmul(
                            tv[:, r * 512:(r + 1) * 512],
                            lhsT=wvT_sb[:, ec, :],
                            rhs=xT_sb[:, ec, base + r * 512:
                                      base + (r + 1) * 512],
                            start=(ec == 0),
                            stop=(ec == EO - 1),
                        )
                for r in range(2):
                    sl = slice(base + r * 512, base + (r + 1) * 512)
                    nc.vector.tensor_scalar_add(
                        vT_sb[:, sl], tv[:, r * 512:(r + 1) * 512], bv_sb[:])
            for h in range(NH_LOC):
                hsl = slice(h * HS, (h + 1) * HS)
                vstg = work.tile([P, SKC, HS], BF, tag="vstg")
                nc.sync.dma_start_transpose(vstg[:], vT_sb[hsl, :])
                nc.vector.tensor_copy(vaug_sb[:, h, :, :HS], vstg[:])

            # ---------- attention ----------
            # ST[sk,sq] = K Q^T / 8 -> P~ = exp; yt = [V+bv | 1]^T P~
            # y = yt[:64] * (1/Z) with Z = yt[64] (includes the Z*bv fold).
            # out[s,f] = sum_e yTn[e,s] wdT[e,f]: each qb's output projection
            # is interleaved into the NEXT qb's attention j-loop so the PE
            # never stalls on the (slow) softmax normalize chain.
            def emit_po(sc, tail=False):
                po = psA.tile([P, SQB], FP, tag="st")
                for r in range(2):
                    rsl = slice(r * 512, (r + 1) * 512)
                    nc.tensor.matmul(
                        po[:, rsl],
                        lhsT=yTn_sb[:, sc * P:(sc + 1) * P],
                        rhs=wdT_sb[:, rsl],
                        start=True,
                        stop=True,
                    )
                ob = outp.tile([P, E], BF, tag="ob")
                if sc % 2 == 0:
                    nc.scalar.copy(ob[:], po[:])
                else:
                    nc.vector.tensor_copy(ob[:], po[:])
                nc.sync.dma_start(out_d[sc * P:(sc + 1) * P, :], ob[:])

            # sq chunks: shrink toward the end so the final normalize +
            # outproj tail covers as little sequence as possible.
            chunks = [(0, 1024), (1024, 1024)]
            prev_po = []
            for (cq0, csz) in chunks:
                qsl = slice(cq0, cq0 + csz)
                for h in range(NH_LOC):
                    # previous chunk's outproj blocks, fed into this j-loop
                    # once its normalize has had time to finish
                    pending = prev_po if h == 0 else []
                    hsl = slice(h * HS, (h + 1) * HS)
                    yt = psB.tile([P, SQB], FP, tag="yt")
                    for j in range(SKC):
                        st = psA.tile([P, SQB], FP, tag="st")
                        for r in range(csz // 512):
                            rsl = slice(r * 512, (r + 1) * 512)
                            nc.tensor.matmul(
                                st[:, rsl],
                                lhsT=kT_sb[hsl, j * P:(j + 1) * P],
                                rhs=qT_sb[hsl, cq0 + r * 512:
                                          cq0 + (r + 1) * 512],
                                start=True,
                                stop=True,
                            )
                        pt = work.tile([P, SQB], BF, tag="pt")
                        nc.scalar.activation(
                            pt[:, :csz], st[:, :csz], AF.Exp, scale=0.125)
                        for r in range(csz // 512):
                            rsl = slice(r * 512, (r + 1) * 512)
                            nc.tensor.matmul(
                                yt[:HS + 1, rsl],
                                lhsT=vaug_sb[:, h, j, :],
                                rhs=pt[:, rsl],
                                start=(j == 0),
                                stop=(j == SKC - 1),
                            )
                        if j >= SKC - len(pending):
                            emit_po(pending[j - (SKC - len(pending))])
                    # normalize: y = yt[:64] / Z  (Z in row 64); 1/Z is DMA
                    # round-trip broadcast across partitions.  The very last
                    # (chunk, h) is the kernel tail: split it into pipelined
                    # halves and chase each half with its outproj blocks so
                    # the PE restarts as early as possible.
                    last = (cq0, h) == (chunks[-1][0], NH_LOC - 1)
                    nsp = 2 if last else 1
                    for half in range(nsp):
                        w = csz // nsp
                        lsl = slice(half * w, (half + 1) * w)
                        gsl = slice(cq0 + half * w, cq0 + (half + 1) * w)
                        zri = nrm.tile([1, SQB], FP, tag="zri")
                        nc.vector.reciprocal(
                            zri[:, lsl], yt[HS:HS + 1, lsl])
                        zrd = drs.tile([1, SQB], FP, tag="zrd")
                        nc.sync.dma_start(zrd[:, lsl], zri[:, lsl])
                        zbs = nrm.tile([HS, SQB], FP, tag="zbs")
                        nc.sync.dma_start(
                            zbs[:, lsl],
                            zrd[0:1, lsl].to_broadcast((HS, w)))
                        nc.vector.tensor_mul(
                            yTn_sb[hsl, gsl], yt[:HS, lsl], zbs[:, lsl])
                        if last:
                            for sc in range((cq0 + half * w) // P,
                                            (cq0 + (half + 1) * w) // P):
                                emit_po(sc, tail=True)
                prev_po = list(range(cq0 // P, (cq0 + csz) // P))

    nc.compile()
    return nc


_NC_CACHE = None


def _get_nc():
    global _NC_CACHE
    if _NC_CACHE is None:
        _NC_CACHE = build_nc()
    return _NC_CACHE


def make_in_maps(x, W_qkv, b_qkv, rotary, W_dense, b_dense):
    x = np.asarray(x, dtype=np.float32)
    W_qkv = np.asarray(W_qkv, dtype=np.float32)
    b_qkv = np.asarray(b_qkv, dtype=np.float32)
    rotary = np.asarray(rotary, dtype=np.float32)
    W_dense = np.asarray(W_dense, dtype=np.float32)

    bf16 = ml_dtypes.bfloat16
    xT = np.ascontiguousarray(x.reshape(S, E).T.astype(bf16))
    wq = W_qkv[0:E, :]            # [E(out f), E(in e)]
    bq = b_qkv[0:E]
    in_maps = []
    for c in range(N_CORES):
        lo, hi = P * c, P * (c + 1)
        rot_c = rotary[:, lo:hi]                    # [E(f), 128(g)]
        wqT_eff = wq.T @ rot_c                      # [E(e), 128(g)]
        bqe = bq @ rot_c                            # [128(g)]
        in_maps.append({
            "xT": xT,
            "wqT": np.ascontiguousarray(wqT_eff.astype(bf16)),
            "wkT": np.ascontiguousarray(W_qkv[E + lo:E + hi, :].T.astype(bf16)),
            "wvT": np.ascontiguousarray(
                W_qkv[2 * E + lo:2 * E + hi, :].T.astype(bf16)),
            "wdT": np.ascontiguousarray(W_dense[:, lo:hi].T.astype(bf16)),
            "bqe": np.ascontiguousarray(bqe),
            "bk": np.ascontiguousarray(b_qkv[E + lo:E + hi]),
            "bv": np.ascontiguousarray(b_qkv[2 * E + lo:2 * E + hi]),
        })
    return in_maps


def run(inputs, trace=False, **trace_kwargs):
    """Run on 8 cores; returns (full_output, BassKernelResults)."""
    nc = _get_nc()
    in_maps = make_in_maps(**inputs)
    br = run_bass_kernel_spmd(
        nc, in_maps, core_ids=list(range(N_CORES)), trace=trace, **trace_kwargs
    )
    b_dense = np.asarray(inputs["b_dense"], dtype=np.float32)
    acc = np.zeros((S, E), dtype=np.float32)
    for r in br.results:
        acc += np.asarray(r["out"], dtype=np.float32)
    acc += b_dense[None, :]
    return acc[None, :, :], br


def kernel(**inputs) -> np.ndarray:
    out, _ = run(inputs, trace=False)
    return out
